# revision 40
# baseline (speedup 1.0000x reference)
"""SuperposedExpert, token-parallel variant: 8-way DP, no collectives.

Each core handles 256 tokens and runs ALL 4 paths over them, accumulating the
gated sum locally in SBUF — no ReduceScatter, so the kernel tail is just the
final output store. TT cores are expanded to dense W1/W2 on the host (with
(1 + path_weight) folded into W2). Per path: W1 (8MB bf16) is double-buffered
resident (prefetch next path during current), W2 is streamed in 1MB slices
through an f-outer ffn2 that keeps all 8 psum banks accumulating.
"""

import numpy as np
import ml_dtypes

import concourse.bass as bass
import concourse.tile as tile
from concourse import bacc, mybir
from concourse.bass import ds, ts
from concourse.bass_utils import run_bass_kernel_spmd

BF16 = mybir.dt.bfloat16
F32 = mybir.dt.float32
AF = mybir.ActivationFunctionType

K = 4
D = 1024
DFF = 4096
NTOK = 2048
NCORES = 8
NTC = NTOK // NCORES   # 256 tokens per core


def _emit(nc, tc):
    xTp = nc.dram_tensor("xTp", [128, 8, NTC], BF16, kind="ExternalInput")
    # W1 packed f-quarter-major: [g][p][s][fq] so each quarter is one
    # contiguous-HBM 2MB DMA and ffn1 group g starts after (g+1) quarters
    w1p = [nc.dram_tensor(f"w1p{k}", [4, 128, 8, DFF // 4], BF16,
                          kind="ExternalInput") for k in range(K)]
    w2p = [nc.dram_tensor(f"w2p{k}", [128, 32, D], BF16, kind="ExternalInput")
           for k in range(K)]
    pbT = nc.dram_tensor("pbT", [D, K], BF16, kind="ExternalInput")
    ones4 = nc.dram_tensor("ones4", [1, K], F32, kind="ExternalInput")
    onesK1d = nc.dram_tensor("onesK1", [K, 1], F32, kind="ExternalInput")
    selbc = nc.dram_tensor("selbc", [K, K * 128], F32, kind="ExternalInput")
    opiece = nc.dram_tensor("opiece", [128, 8, NTC], BF16, kind="ExternalOutput")
    # 16-byte scratch: a read-dependency gate for the scalar-ring stream
    gate_d = nc.dram_tensor("gate_d", [1, 8], BF16)

    with (
        tc.tile_pool(name="w1pool", bufs=8) as w1pool,
        tc.tile_pool(name="w2pool", bufs=8) as w2pool,
        tc.tile_pool(name="small", bufs=1) as small,
        tc.tile_pool(name="obp", bufs=4) as obp,
        tc.tile_pool(name="pp", bufs=8, space="PSUM") as pp,
    ):
        xt_sb = small.tile([128, 8, NTC], BF16, tag="xt")
        nc.sync.dma_start(xt_sb, xTp.ap())
        pbt_sb = small.tile([128, 8, K], BF16, tag="pbt")
        nc.sync.dma_start(pbt_sb, pbT.ap().rearrange("(t p) k -> p t k", p=128))
        ones4_sb = small.tile([1, K], F32, tag="ones4")
        nc.sync.dma_start(ones4_sb, ones4.ap())
        selbc_sb = small.tile([K, K * 128], F32, tag="selbc")
        nc.sync.dma_start(selbc_sb, selbc.ap())

        expl = small.tile([K, NTC], F32, tag="expl")
        gall = small.tile([K, NTC], F32, tag="gall")
        rden4 = small.tile([K, NTC], F32, tag="rden4")
        gbc4 = small.tile([128, K, NTC], F32, tag="gbc4")
        acc = small.tile([128, 8, NTC], F32, tag="acc")

        # w1 as independent per-quarter tiles [p, s, fq] matching the DRAM
        # packing (quarter DMAs contiguous on both sides, no same-tile
        # serialization); path k resident while k+1 prefetches (8 slots).
        # Path 0's load rides the sync ring: the scalar engine's ~10us
        # preamble would delay the critical first quarters.
        w1q = [[w1pool.tile([128, 8, DFF // 4], BF16, tag="w1",
                            name=f"w1_{k}_{g}") for g in range(4)]
               for k in range(K)]
        for g in range(4):
            nc.sync.dma_start(w1q[0][g], w1p[0][g])

        onesK1 = small.tile([K, 1], F32, tag="onesK1")
        nc.sync.dma_start(onesK1, onesK1d.ap())

        # --- gating: logits^T [K, n] = pbT^T @ xT; softmax over K ---
        lps = pp.tile([K, NTC], F32, tag="ps", name="gl")
        for kc in range(8):
            nc.tensor.matmul(lps, pbt_sb[:, kc], xt_sb[:, kc],
                             start=(kc == 0), stop=(kc == 7))
        nc.scalar.activation(expl, lps, AF.Exp)
        den = pp.tile([1, NTC], F32, tag="ps", name="gd")
        nc.tensor.matmul(den, onesK1, expl)         # [1, n] sum over k
        rden1 = small.tile([1, NTC], F32, tag="rden1")
        nc.vector.reciprocal(rden1, den)
        r4ps = pp.tile([K, NTC], F32, tag="ps", name="gr")
        nc.tensor.matmul(r4ps, ones4_sb, rden1)     # broadcast to K partitions
        nc.vector.tensor_copy(rden4, r4ps)
        nc.vector.tensor_mul(gall, expl, rden4)     # gates [K, n]
        for k in range(K):
            gps = pp.tile([128, NTC], F32, tag="ps", name=f"gb_{k}")
            nc.tensor.matmul(gps, selbc_sb[:, ts(k, 128)], gall)
            nc.vector.tensor_copy(gbc4[:, k], gps)

        # ---------------- per-path FFN ----------------
        # All weight streams share the scalar HWDGE ring, emitted in exact
        # consumption order, so ring-FIFO paces HBM deterministically:
        # per path-cycle ~8MB W1 + 8MB W2 spread evenly across both windows.
        for k in range(K):
            ht = small.tile([128, 32, NTC], BF16, tag="ht", name=f"ht_{k}")
            w2tl = [None] * 16

            def w2_load(q):
                w2tl[q] = w2pool.tile([128, 2, D], BF16, tag="w2",
                                      name=f"w2_{k}_{q}")
                nc.scalar.dma_start(w2tl[q], w2p[k][:, ds(2 * q, 2), :])

            for grp in range(4):
                ps1 = [pp.tile([128, NTC], F32, tag="ps",
                               name=f"f1_{k}_{grp}_{j}") for j in range(8)]
                for s in range(8):
                    for j in range(8):
                        nc.tensor.matmul(
                            ps1[j], w1q[k][grp][:, s, ts(j, 128)],
                            xt_sb[:, s],
                            start=(s == 0), stop=(s == 7),
                        )
                for j in range(8):
                    nc.scalar.activation(ht[:, grp * 8 + j], ps1[j],
                                         AF.Gelu_apprx_tanh)
                # prefetches after the gelus; for path 0 the whole scalar
                # stream is held behind a tiny DMA that READS a gelu output,
                # so early HBM stays exclusive to path 0's critical W1 load
                # (the ring is FIFO, so one gate holds everything behind it)
                if k == 0:
                    if grp == 1:
                        nc.scalar.dma_start(
                            gate_d.ap(), ht[ds(0, 1), 8, ds(0, 8)])
                        nc.scalar.dma_start(w1q[1][0], w1p[1][0])
                        for q in range(4):
                            w2_load(q)
                        nc.scalar.dma_start(w1q[1][1], w1p[1][1])
                    elif grp >= 2:
                        w2_load(2 * grp)
                        w2_load(2 * grp + 1)
                else:
                    if k + 1 < K and grp < 2:
                        nc.scalar.dma_start(w1q[k + 1][grp], w1p[k + 1][grp])
                    w2_load(2 * grp)
                    w2_load(2 * grp + 1)

            # ffn2 f-outer: all 8 d-tiles accumulate across the 32 f-chunks
            ps2 = [pp.tile([128, NTC], F32, tag="ps", name=f"f2_{k}_{m2}")
                   for m2 in range(8)]
            for q in range(16):
                if q < 8:
                    if k + 1 < K and q in (2, 6):
                        g = 2 + (q == 6)
                        nc.scalar.dma_start(w1q[k + 1][g], w1p[k + 1][g])
                    w2_load(q + 8)
                for kc in range(2):
                    f = 2 * q + kc
                    for m2 in range(8):
                        nc.tensor.matmul(
                            ps2[m2], w2tl[q][:, kc, ts(m2, 128)], ht[:, f],
                            start=(f == 0), stop=(f == 31),
                        )
            for m2 in range(8):
                if k == 0:
                    nc.vector.tensor_mul(acc[:, m2], ps2[m2], gbc4[:, k])
                else:
                    ob = obp.tile([128, NTC], F32, tag="ob",
                                  name=f"ob_{k}_{m2}")
                    nc.vector.tensor_mul(ob, ps2[m2], gbc4[:, k])
                    if k < K - 1:
                        nc.vector.tensor_add(acc[:, m2], acc[:, m2], ob)
                    else:
                        obf = obp.tile([128, NTC], BF16, tag="obf",
                                       name=f"obf_{m2}")
                        nc.vector.tensor_add(obf, acc[:, m2], ob)
                        nc.sync.dma_start(opiece[:, m2, :], obf)


def build(verbose=False):
    nc = bacc.Bacc("TRN2", target_bir_lowering=False, debug=False, num_devices=NCORES)
    with tile.TileContext(nc) as tc:
        _emit(nc, tc)
    nc.compile()
    return nc


def _expand_tt(core1, core2, din, dout):
    a, x, r = core1.shape
    r2, b, y = core2.shape
    m = core1.reshape(a * x, r).astype(np.float32) @ \
        core2.reshape(r2, b * y).astype(np.float32)
    w = m.reshape(a, x, b, y).transpose(0, 2, 1, 3).reshape(a * b, x * y)
    assert w.shape == (din, dout)
    return w


def make_in_maps(inputs):
    tokens = inputs["tokens"]
    bf = ml_dtypes.bfloat16
    shared = {}
    for k in range(K):
        w1 = _expand_tt(inputs["ffn1_core1"][k], inputs["ffn1_core2"][k], D, DFF)
        # [(s p), (g fq)] -> [g, p, s, fq]
        shared[f"w1p{k}"] = np.ascontiguousarray(
            w1.reshape(8, 128, 4, DFF // 4).transpose(2, 1, 0, 3)).astype(bf)
        w2 = _expand_tt(inputs["ffn2_core1"][k], inputs["ffn2_core2"][k], DFF, D)
        w2 *= (1.0 + inputs["path_weights"][k])[None, :]
        shared[f"w2p{k}"] = np.ascontiguousarray(
            w2.reshape(32, 128, D).transpose(1, 0, 2)).astype(bf)
    shared["pbT"] = np.ascontiguousarray(inputs["path_bases"].T).astype(bf)
    shared["ones4"] = np.ones((1, K), np.float32)
    shared["onesK1"] = np.ones((K, 1), np.float32)
    selbc = np.zeros((K, K * 128), np.float32)
    for k in range(K):
        selbc[k, k * 128:(k + 1) * 128] = 1.0
    shared["selbc"] = selbc
    in_maps = []
    for c in range(NCORES):
        tok = tokens[c * NTC:(c + 1) * NTC]
        xt = np.ascontiguousarray(
            tok.T.reshape(8, 128, NTC).transpose(1, 0, 2)).astype(bf)
        m = dict(shared)
        m["xTp"] = xt
        in_maps.append(m)
    return in_maps


def assemble(results):
    out = np.empty((NTOK, D), np.float32)
    for c in range(NCORES):
        # [128 p, 8 t, 256 n]; d = t*128+p
        piece = results[c]["opiece"].astype(np.float32)
        out[c * NTC:(c + 1) * NTC] = piece.transpose(2, 1, 0).reshape(NTC, D)
    return out


_NC = None


def run(inputs, trace=False):
    global _NC
    if _NC is None:
        _NC = build()
    res = run_bass_kernel_spmd(
        _NC, make_in_maps(inputs), core_ids=list(range(NCORES)), trace=trace
    )
    return assemble(res.results), res


def kernel(**inputs):
    out, _ = run(inputs)
    return out


# revision 45
# speedup vs baseline: 1.0684x; 1.0684x over previous
"""SuperposedExpert, token-parallel variant: 8-way DP, no collectives.

Each core handles 256 tokens and runs ALL 4 paths over them, accumulating the
gated sum locally in SBUF — no ReduceScatter, so the kernel tail is just the
final output store. TT cores are expanded to dense W1/W2 on the host (with
(1 + path_weight) folded into W2). Per path: W1 (8MB bf16) is double-buffered
resident (prefetch next path during current), W2 is streamed in 1MB slices
through an f-outer ffn2 that keeps all 8 psum banks accumulating.
"""

import numpy as np
import ml_dtypes

import concourse.bass as bass
import concourse.tile as tile
from concourse import bacc, mybir
from concourse.bass import ds, ts
from concourse.bass_utils import run_bass_kernel_spmd

BF16 = mybir.dt.bfloat16
F32 = mybir.dt.float32
AF = mybir.ActivationFunctionType

K = 4
D = 1024
DFF = 4096
NTOK = 2048
NCORES = 8
NTC = NTOK // NCORES   # 256 tokens per core


def _emit(nc, tc):
    xTp = nc.dram_tensor("xTp", [128, 8, NTC], BF16, kind="ExternalInput")
    # W1 packed f-quarter-major: [g][p][s][fq] so each quarter is one
    # contiguous-HBM 2MB DMA and ffn1 group g starts after (g+1) quarters
    w1p = [nc.dram_tensor(f"w1p{k}", [4, 128, 8, DFF // 4], BF16,
                          kind="ExternalInput") for k in range(K)]
    w2p = [nc.dram_tensor(f"w2p{k}", [128, 32, D], BF16, kind="ExternalInput")
           for k in range(K)]
    pbT = nc.dram_tensor("pbT", [D, K], BF16, kind="ExternalInput")
    ones4 = nc.dram_tensor("ones4", [1, K], F32, kind="ExternalInput")
    onesK1d = nc.dram_tensor("onesK1", [K, 1], F32, kind="ExternalInput")
    selbc = nc.dram_tensor("selbc", [K, K * 128], F32, kind="ExternalInput")
    opiece = nc.dram_tensor("opiece", [128, 8, NTC], BF16, kind="ExternalOutput")


    with (
        # bufs are the pacing mechanism: a prefetch DMA can only start once
        # its slot's previous tenant has been fully consumed by the PE, so
        # weight streams self-schedule just-in-time without ordering hacks.
        tc.tile_pool(name="w1pool", bufs=4) as w1pool,
        tc.tile_pool(name="w2pool", bufs=8) as w2pool,
        tc.tile_pool(name="small", bufs=1) as small,
        tc.tile_pool(name="obp", bufs=4) as obp,
        tc.tile_pool(name="pp", bufs=8, space="PSUM") as pp,
    ):
        xt_sb = small.tile([128, 8, NTC], BF16, tag="xt")
        nc.sync.dma_start(xt_sb, xTp.ap())
        pbt_sb = small.tile([128, 8, K], BF16, tag="pbt")
        nc.sync.dma_start(pbt_sb, pbT.ap().rearrange("(t p) k -> p t k", p=128))
        ones4_sb = small.tile([1, K], F32, tag="ones4")
        nc.sync.dma_start(ones4_sb, ones4.ap())
        selbc_sb = small.tile([K, K * 128], F32, tag="selbc")
        nc.sync.dma_start(selbc_sb, selbc.ap())

        expl = small.tile([K, NTC], F32, tag="expl")
        gall = small.tile([K, NTC], F32, tag="gall")
        rden4 = small.tile([K, NTC], F32, tag="rden4")
        gbc4 = small.tile([128, K, NTC], F32, tag="gbc4")
        acc = small.tile([128, 8, NTC], F32, tag="acc")

        # w1 as independent per-quarter tiles [p, s, fq] matching the DRAM
        # packing (quarter DMAs contiguous on both sides, no same-tile
        # serialization); path k resident while k+1 prefetches (8 slots).
        # Path 0's load rides the sync ring: the scalar engine's ~10us
        # preamble would delay the critical first quarters.
        w1q = [[w1pool.tile([128, 8, DFF // 4], BF16, tag="w1",
                            name=f"w1_{k}_{g}") for g in range(4)]
               for k in range(K)]
        for g in range(4):
            nc.sync.dma_start(w1q[0][g], w1p[0][g])

        onesK1 = small.tile([K, 1], F32, tag="onesK1")
        nc.sync.dma_start(onesK1, onesK1d.ap())

        # --- gating: logits^T [K, n] = pbT^T @ xT; softmax over K ---
        lps = pp.tile([K, NTC], F32, tag="ps", name="gl")
        for kc in range(8):
            nc.tensor.matmul(lps, pbt_sb[:, kc], xt_sb[:, kc],
                             start=(kc == 0), stop=(kc == 7))
        nc.scalar.activation(expl, lps, AF.Exp)
        den = pp.tile([1, NTC], F32, tag="ps", name="gd")
        nc.tensor.matmul(den, onesK1, expl)         # [1, n] sum over k
        rden1 = small.tile([1, NTC], F32, tag="rden1")
        nc.vector.reciprocal(rden1, den)
        r4ps = pp.tile([K, NTC], F32, tag="ps", name="gr")
        nc.tensor.matmul(r4ps, ones4_sb, rden1)     # broadcast to K partitions
        nc.vector.tensor_copy(rden4, r4ps)
        nc.vector.tensor_mul(gall, expl, rden4)     # gates [K, n]
        for k in range(K):
            gps = pp.tile([128, NTC], F32, tag="ps", name=f"gb_{k}")
            nc.tensor.matmul(gps, selbc_sb[:, ts(k, 128)], gall)
            nc.vector.tensor_copy(gbc4[:, k], gps)

        # ---------------- per-path FFN ----------------
        # All weight streams share the scalar HWDGE ring, emitted in exact
        # consumption order, so ring-FIFO paces HBM deterministically:
        # per path-cycle ~8MB W1 + 8MB W2 spread evenly across both windows.
        for k in range(K):
            ht = small.tile([128, 32, NTC], BF16, tag="ht", name=f"ht_{k}")
            w2tl = [None] * 16

            def w2_load(q):
                w2tl[q] = w2pool.tile([128, 2, D], BF16, tag="w2",
                                      name=f"w2_{k}_{q}")
                # path 0's first slices ride the sync ring FIFO-behind the
                # critical W1 load; everything later is slot-WAR paced
                eng = nc.sync if (k == 0 and q < 8) else nc.scalar
                eng.dma_start(w2tl[q], w2p[k][:, ds(2 * q, 2), :])

            for grp in range(4):
                ps1 = [pp.tile([128, NTC], F32, tag="ps",
                               name=f"f1_{k}_{grp}_{j}") for j in range(8)]
                for s in range(8):
                    for j in range(8):
                        nc.tensor.matmul(
                            ps1[j], w1q[k][grp][:, s, ts(j, 128)],
                            xt_sb[:, s],
                            start=(s == 0), stop=(s == 7),
                        )
                for j in range(8):
                    nc.scalar.activation(ht[:, grp * 8 + j], ps1[j],
                                         AF.Gelu_apprx_tanh)
                if k + 1 < K:
                    nc.scalar.dma_start(w1q[k + 1][grp], w1p[k + 1][grp])
                w2_load(2 * grp)
                w2_load(2 * grp + 1)

            # ffn2 f-outer: all 8 d-tiles accumulate across the 32 f-chunks
            ps2 = [pp.tile([128, NTC], F32, tag="ps", name=f"f2_{k}_{m2}")
                   for m2 in range(8)]
            for q in range(16):
                if q < 8:
                    w2_load(q + 8)
                for kc in range(2):
                    f = 2 * q + kc
                    for m2 in range(8):
                        nc.tensor.matmul(
                            ps2[m2], w2tl[q][:, kc, ts(m2, 128)], ht[:, f],
                            start=(f == 0), stop=(f == 31),
                        )
            for m2 in range(8):
                if k == 0:
                    nc.vector.tensor_mul(acc[:, m2], ps2[m2], gbc4[:, k])
                else:
                    ob = obp.tile([128, NTC], F32, tag="ob",
                                  name=f"ob_{k}_{m2}")
                    nc.vector.tensor_mul(ob, ps2[m2], gbc4[:, k])
                    if k < K - 1:
                        nc.vector.tensor_add(acc[:, m2], acc[:, m2], ob)
                    else:
                        obf = obp.tile([128, NTC], BF16, tag="obf",
                                       name=f"obf_{m2}")
                        nc.vector.tensor_add(obf, acc[:, m2], ob)
                        nc.sync.dma_start(opiece[:, m2, :], obf)


def build(verbose=False):
    nc = bacc.Bacc("TRN2", target_bir_lowering=False, debug=False, num_devices=NCORES)
    with tile.TileContext(nc) as tc:
        _emit(nc, tc)
    nc.compile()
    return nc


def _expand_tt(core1, core2, din, dout):
    a, x, r = core1.shape
    r2, b, y = core2.shape
    m = core1.reshape(a * x, r).astype(np.float32) @ \
        core2.reshape(r2, b * y).astype(np.float32)
    w = m.reshape(a, x, b, y).transpose(0, 2, 1, 3).reshape(a * b, x * y)
    assert w.shape == (din, dout)
    return w


def make_in_maps(inputs):
    tokens = inputs["tokens"]
    bf = ml_dtypes.bfloat16
    shared = {}
    for k in range(K):
        w1 = _expand_tt(inputs["ffn1_core1"][k], inputs["ffn1_core2"][k], D, DFF)
        # [(s p), (g fq)] -> [g, p, s, fq]
        shared[f"w1p{k}"] = np.ascontiguousarray(
            w1.reshape(8, 128, 4, DFF // 4).transpose(2, 1, 0, 3)).astype(bf)
        w2 = _expand_tt(inputs["ffn2_core1"][k], inputs["ffn2_core2"][k], DFF, D)
        w2 *= (1.0 + inputs["path_weights"][k])[None, :]
        shared[f"w2p{k}"] = np.ascontiguousarray(
            w2.reshape(32, 128, D).transpose(1, 0, 2)).astype(bf)
    shared["pbT"] = np.ascontiguousarray(inputs["path_bases"].T).astype(bf)
    shared["ones4"] = np.ones((1, K), np.float32)
    shared["onesK1"] = np.ones((K, 1), np.float32)
    selbc = np.zeros((K, K * 128), np.float32)
    for k in range(K):
        selbc[k, k * 128:(k + 1) * 128] = 1.0
    shared["selbc"] = selbc
    in_maps = []
    for c in range(NCORES):
        tok = tokens[c * NTC:(c + 1) * NTC]
        xt = np.ascontiguousarray(
            tok.T.reshape(8, 128, NTC).transpose(1, 0, 2)).astype(bf)
        m = dict(shared)
        m["xTp"] = xt
        in_maps.append(m)
    return in_maps


def assemble(results):
    out = np.empty((NTOK, D), np.float32)
    for c in range(NCORES):
        # [128 p, 8 t, 256 n]; d = t*128+p
        piece = results[c]["opiece"].astype(np.float32)
        out[c * NTC:(c + 1) * NTC] = piece.transpose(2, 1, 0).reshape(NTC, D)
    return out


_NC = None


def run(inputs, trace=False):
    global _NC
    if _NC is None:
        _NC = build()
    res = run_bass_kernel_spmd(
        _NC, make_in_maps(inputs), core_ids=list(range(NCORES)), trace=trace
    )
    return assemble(res.results), res


def kernel(**inputs):
    out, _ = run(inputs)
    return out
